# revision 7
# baseline (speedup 1.0000x reference)
"""Linear attention Bass kernel for Trainium2 (8 NeuronCores).

Problem: x [4, 8192, 1024] f32, W [1024, 3072] f32.
  qkv = x @ W; q,k,v = split(qkv); q,k = elu(.)+1
  KV = einsum('bld,blh->bhd', k, v); ksum = k.sum(1)
  Z = 1/(q.ksum + eps); V = einsum('bld,bhd,bl->blh', q, KV, Z)

Sharding: 8 cores, core c handles batch b=c//2, sequence half h=c%2
(4096 rows each).  KV / ksum reductions span the full batch sequence, so
the two cores of a pair AllReduce their partial KV^T [1024,1024] + ksum
(4.2 MB fp32) in-NEFF.  Fallback (USE_CC=False): each core redundantly
computes k,v for the sibling half (no collectives) and receives full W.

Under axon the dominant cost is host<->device PJRT traffic over the
tunnel (~75-110 MB/s, serialized), not on-chip time.  Traffic diet vs
the first working version:
  * W is shipped once as per-core [128, 3072] slices (6.3 MB total
    instead of 8x duplicated 50 MB) and AllGathered on-chip across all
    8 cores, then laid out into SBUF.
  * the output is int8 with a per-row f32 dequant scale (quarters both
    the donated zero-output upload and the result download vs f32).
    V_row = z * pv_row with z > 0, so int8 = pv * (127/rowmax|pv|) and
    scale = rowmax|pv| * z / 127; the host computes i8 * scale.

Per-core dataflow (all matmuls bf16 inputs, fp32 PSUM accumulation):
  phase 0: DMA w_slice -> SBUF -> internal DRAM; AllGather[0..7] ->
           full W [1024, 3072] on every core; 16 strided DMAs lay it
           out as wq_sb [128,8,1024] / wkv_sb [128,8,2048].
  phase 1: stream xT tiles; q^T = Wq^T-form matmul (comes out [d,l] ready
           for phase 3), k,v = standard form [l,d]; phi=elu+1 via
           exp/min/max; q^T -> DRAM stash, k,v -> DRAM stash;
           ksum accumulated in PSUM via ones-vector matmul.
  phase 2: KV^T[d,h] += k_tile^T-free matmul over all l chunks, h in two
           512 halves (PSUM = 8 banks per half); partial KV^T + ksum ->
           cc buffer; AllReduce over core pairs.
  phase 3: V[l,:] = (q^T)^T @ KV^T, denominator from ksum column matmul,
           z = 1/(den+eps), scale, bf16 DMA out.
"""

import numpy as np
import ml_dtypes

import concourse.bass as bass
import concourse.tile as tile
from concourse import mybir
from concourse.bacc import Bacc

USE_CC = True
TRACE = False
LAST_RESULTS = None

B, L, D = 4, 8192, 1024
NCORES = 8
R = 4096              # rows per core
LT = 512              # l-tile width (columns of xT per tile)
EPS = 1e-6

BF16 = mybir.dt.bfloat16
F32 = mybir.dt.float32
NPBF16 = ml_dtypes.bfloat16

_NC_CACHE = {}


def _emit_phi(nc, pool_e, out_bf, psum_in, width):
    """out_bf (bf16) = elu(psum_in)+1 = min(exp(y),1) + max(y,0).

    Ops are emitted per 512-wide slice so each reads a single PSUM bank
    (one stop-matmul dep); the combine reads only SBUF tiles.  Keeps the
    per-instruction semaphore-wait count under the ISA limit.
    """
    for s in range(0, width, 512):
        w = min(512, width - s)
        ps = psum_in[:, s : s + w]
        e = pool_e.tile([128, w], F32, tag=f"phi_e_{w}_{s}", name=f"e{w}_{s}")
        nc.scalar.activation(out=e, in_=ps, func=mybir.ActivationFunctionType.Exp)
        r = pool_e.tile([128, w], F32, tag=f"phi_r_{w}_{s}", name=f"r{w}_{s}")
        nc.vector.tensor_scalar(
            out=r, in0=ps, scalar1=0.0, scalar2=None, op0=mybir.AluOpType.max
        )
        nc.vector.scalar_tensor_tensor(
            out=out_bf[:, s : s + w],
            in0=e,
            scalar=1.0,
            in1=r,
            op0=mybir.AluOpType.min,
            op1=mybir.AluOpType.add,
        )


def build_bass(use_cc=USE_CC):
    nc = Bacc(trn_type="TRN2", num_devices=NCORES)

    n_xt_cols = R if use_cc else 2 * R
    n_lc = n_xt_cols // 128          # 32 or 64 chunks of 128 rows
    n_tiles = n_xt_cols // LT        # 8 or 16 l-tiles
    local_tiles = R // LT            # 8 tiles that produce q/output

    xt = nc.dram_tensor("xt", [128, 8, n_xt_cols], BF16, kind="ExternalInput")
    if use_cc:
        # each core brings 1/8 of W; AllGather reconstructs the full W
        w_slice = nc.dram_tensor("w", [128, 3072], BF16, kind="ExternalInput")
        w_in = nc.dram_tensor("w_in", [128, 3072], BF16)
        w_all = nc.dram_tensor("w_all", [1024, 3072], BF16)
    else:
        w_all = nc.dram_tensor("w", [1024, 3072], BF16, kind="ExternalInput")
    out = nc.dram_tensor("out", [R, 1024], mybir.dt.int8, kind="ExternalOutput")
    out_sc = nc.dram_tensor("out_sc", [R, 1], F32, kind="ExternalOutput")

    q_dram = nc.dram_tensor("q_stash", [128, 8, R], BF16)
    k_dram = nc.dram_tensor("k_stash", [n_lc, 128, 1024], BF16)
    v_dram = nc.dram_tensor("v_stash", [n_lc, 128, 1024], BF16)
    if use_cc:
        # row 128 of each [129, 1024] chunk holds ksum[m*128:(m+1)*128] in
        # cols 0:128 (rest zeros, harmlessly allreduced).
        cc_in = nc.dram_tensor("cc_in", [8, 129, 1024], F32)
        cc_out = nc.dram_tensor("cc_out", [8, 129, 1024], F32)
    else:
        ks_dram = nc.dram_tensor("ks_stash", [8, 128], F32)

    mm = nc.tensor.matmul
    Act = mybir.ActivationFunctionType

    with tile.TileContext(nc) as tc:
        with tc.tile_pool(name="consts", bufs=1) as consts:
            # ---------------- phase 0: W distribution ---------------------
            if use_cc:
                w_sb = consts.tile([128, 3072], BF16)
                nc.sync.dma_start(out=w_sb, in_=w_slice[:])
                nc.sync.dma_start(out=w_in[:], in_=w_sb)
                nc.gpsimd.collective_compute(
                    "AllGather",
                    mybir.AluOpType.bypass,
                    replica_groups=[list(range(NCORES))],
                    ins=[w_in[:]],
                    outs=[w_all[:]],
                )
            wq_sb = consts.tile([128, 8, 1024], BF16)
            wkv_sb = consts.tile([128, 8, 2048], BF16)
            for k in range(8):
                nc.sync.dma_start(
                    out=wq_sb[:, k, :], in_=w_all[k * 128 : (k + 1) * 128, 0:1024]
                )
                nc.sync.dma_start(
                    out=wkv_sb[:, k, :],
                    in_=w_all[k * 128 : (k + 1) * 128, 1024:3072],
                )
            ones_sb = consts.tile([128, 1], BF16)
            nc.vector.memset(ones_sb, 1.0)

            # ---------------- phase 1: qkv + phi + stashes + ksum ---------
            with (
                tc.tile_pool(name="xt_p", bufs=3) as xt_p,
                tc.tile_pool(name="qout_p", bufs=2) as qout_p,
                tc.tile_pool(name="e_p", bufs=4) as e_p,
                tc.tile_pool(name="kt_p", bufs=3) as kt_p,
                tc.tile_pool(name="vt_p", bufs=3) as vt_p,
                tc.tile_pool(name="q_ps_p", bufs=2, space="PSUM") as q_ps_p,
                tc.tile_pool(name="kv_ps_p", bufs=1, space="PSUM") as kv_ps_p,
                tc.tile_pool(name="ks_ps_p", bufs=1, space="PSUM") as ks_ps_p,
            ):
                ksum_ps = [
                    ks_ps_p.tile([1, 512], F32, tag=f"ks{h}", name=f"ks{h}")
                    for h in range(2)
                ]

                def q_block(xt_tile, qout, m):
                    pq = q_ps_p.tile([128, LT], F32)
                    for k in range(8):
                        mm(
                            pq,
                            lhsT=wq_sb[:, k, m * 128 : (m + 1) * 128],
                            rhs=xt_tile[:, k, :],
                            start=(k == 0),
                            stop=(k == 7),
                        )
                    _emit_phi(nc, e_p, qout[:, m, :], pq, LT)

                def kv_block(xt_tile, t, lc):
                    idx = t * 4 + lc
                    # four independent single-bank PSUM tiles: each reader
                    # then carries exactly one stop-matmul dependency.
                    pkv = [
                        kv_ps_p.tile([128, 512], F32, tag=f"pkv{n}", name=f"pkv{n}")
                        for n in range(4)
                    ]
                    for k in range(8):
                        lhsT = xt_tile[:, k, lc * 128 : (lc + 1) * 128]
                        for n in range(4):
                            mm(
                                pkv[n],
                                lhsT=lhsT,
                                rhs=wkv_sb[:, k, n * 512 : (n + 1) * 512],
                                start=(k == 0),
                                stop=(k == 7),
                            )
                    kt = kt_p.tile([128, 1024], BF16)
                    for s in range(2):
                        _emit_phi(nc, e_p, kt[:, s * 512 : (s + 1) * 512], pkv[s], 512)
                    vt = vt_p.tile([128, 1024], BF16)
                    for s in range(2):
                        nc.scalar.activation(
                            out=vt[:, s * 512 : (s + 1) * 512],
                            in_=pkv[2 + s],
                            func=Act.Copy,
                        )
                    nc.sync.dma_start(out=k_dram[idx], in_=kt)
                    nc.sync.dma_start(out=v_dram[idx], in_=vt)
                    for h in range(2):
                        mm(
                            ksum_ps[h],
                            lhsT=ones_sb,
                            rhs=kt[:, h * 512 : (h + 1) * 512],
                            start=(idx == 0),
                            stop=(idx == n_lc - 1),
                        )

                for t in range(n_tiles):
                    xt_tile = xt_p.tile([128, 8, LT], BF16)
                    nc.sync.dma_start(
                        out=xt_tile, in_=xt[:, :, t * LT : (t + 1) * LT]
                    )
                    if t < local_tiles:
                        qout = qout_p.tile([128, 8, LT], BF16)
                        for seg in range(4):
                            q_block(xt_tile, qout, 2 * seg)
                            q_block(xt_tile, qout, 2 * seg + 1)
                            kv_block(xt_tile, t, seg)
                        nc.sync.dma_start(
                            out=q_dram[:, :, t * LT : (t + 1) * LT], in_=qout
                        )
                    else:
                        for lc in range(4):
                            kv_block(xt_tile, t, lc)

                # stash ksum (psum) to DRAM before phase-1 psum pools close
                ks_sb = consts.tile([1, 1024], F32)
                for h in range(2):
                    nc.vector.tensor_copy(
                        out=ks_sb[:, h * 512 : (h + 1) * 512], in_=ksum_ps[h]
                    )
                for m in range(8):
                    src = ks_sb[0:1, m * 128 : (m + 1) * 128]
                    if use_cc:
                        nc.sync.dma_start(out=cc_in[m, 128, 0:128], in_=src)
                    else:
                        nc.sync.dma_start(out=ks_dram[m, :], in_=src)

            # ---------------- phase 2: KV^T accumulation ------------------
            with tc.tile_pool(name="p23", bufs=1) as p23:
                if not use_cc:
                    kvt_bf = p23.tile([128, 8, 1024], BF16)
                with (
                    tc.tile_pool(name="k2_p", bufs=6) as k2_p,
                    tc.tile_pool(name="v2_p", bufs=6) as v2_p,
                    tc.tile_pool(name="kvt_ps_p", bufs=1, space="PSUM") as kvt_ps_p,
                ):
                    for half in range(2):
                        kvt_ps = [
                            kvt_ps_p.tile(
                                [128, 512], F32, tag=f"kvt{m}", name=f"kvt{m}"
                            )
                            for m in range(8)
                        ]
                        for lc in range(n_lc):
                            kt2 = k2_p.tile([128, 1024], BF16)
                            nc.sync.dma_start(out=kt2, in_=k_dram[lc])
                            vt2 = v2_p.tile([128, 512], BF16)
                            nc.sync.dma_start(
                                out=vt2,
                                in_=v_dram[lc][:, half * 512 : (half + 1) * 512],
                            )
                            for m in range(8):
                                mm(
                                    kvt_ps[m],
                                    lhsT=kt2[:, m * 128 : (m + 1) * 128],
                                    rhs=vt2,
                                    start=(lc == 0),
                                    stop=(lc == n_lc - 1),
                                )
                        for m in range(8):
                            if use_cc:
                                kvs = k2_p.tile(
                                    [128, 512], F32, tag="kvs", name=f"kvs{half}_{m}"
                                )
                                nc.scalar.activation(
                                    out=kvs, in_=kvt_ps[m], func=Act.Copy
                                )
                                nc.sync.dma_start(
                                    out=cc_in[
                                        m, 0:128, half * 512 : (half + 1) * 512
                                    ],
                                    in_=kvs,
                                )
                            else:
                                nc.vector.tensor_copy(
                                    out=kvt_bf[:, m, half * 512 : (half + 1) * 512],
                                    in_=kvt_ps[m],
                                )

                if use_cc:
                    nc.gpsimd.collective_compute(
                        "AllReduce",
                        mybir.AluOpType.add,
                        replica_groups=[[0, 1], [2, 3], [4, 5], [6, 7]],
                        ins=[cc_in[:]],
                        outs=[cc_out[:]],
                    )

                # ---------------- phase 3: output -------------------------
                with (
                    tc.tile_pool(name="p3", bufs=1) as p3,
                    tc.tile_pool(name="qt_p", bufs=2) as qt_p,
                    tc.tile_pool(name="ob_p", bufs=3) as ob_p,
                    tc.tile_pool(name="z_p", bufs=4) as z_p,
                    tc.tile_pool(name="pv_ps_p", bufs=2, space="PSUM") as pv_ps_p,
                    tc.tile_pool(name="pd_ps_p", bufs=2, space="PSUM") as pd_ps_p,
                ):
                    if use_cc:
                        kvt_f = p3.tile([128, 8, 1024], F32)
                        for m in range(8):
                            nc.sync.dma_start(
                                out=kvt_f[:, m, :], in_=cc_out[m, 0:128, :]
                            )
                        kvt_bf = p3.tile([128, 8, 1024], BF16)
                        for m in range(8):
                            nc.vector.tensor_copy(
                                out=kvt_bf[:, m, :], in_=kvt_f[:, m, :]
                            )
                    ksum_f = p3.tile([128, 8], F32)
                    for m in range(8):
                        if use_cc:
                            nc.sync.dma_start(
                                out=ksum_f[:, m : m + 1], in_=cc_out[m, 128, 0:128]
                            )
                        else:
                            nc.sync.dma_start(
                                out=ksum_f[:, m : m + 1], in_=ks_dram[m, :]
                            )
                    ksum_b = p3.tile([128, 8], BF16)
                    for m in range(8):
                        nc.vector.tensor_copy(
                            out=ksum_b[:, m : m + 1], in_=ksum_f[:, m : m + 1]
                        )

                    for g in range(8):
                        qt = qt_p.tile([128, 8, 512], BF16)
                        nc.sync.dma_start(
                            out=qt, in_=q_dram[:, :, g * 512 : (g + 1) * 512]
                        )
                        for lc in range(4):
                            pv0 = pv_ps_p.tile([128, 512], F32, tag="pv0")
                            pv1 = pv_ps_p.tile([128, 512], F32, tag="pv1")
                            pd = pd_ps_p.tile([128, 1], F32)
                            for k in range(8):
                                lhsT = qt[:, k, lc * 128 : (lc + 1) * 128]
                                st, sp = (k == 0), (k == 7)
                                mm(pv0, lhsT=lhsT, rhs=kvt_bf[:, k, 0:512],
                                   start=st, stop=sp)
                                mm(pv1, lhsT=lhsT, rhs=kvt_bf[:, k, 512:1024],
                                   start=st, stop=sp)
                                mm(pd, lhsT=lhsT, rhs=ksum_b[:, k : k + 1],
                                   start=st, stop=sp)
                            z = z_p.tile([128, 1], F32)
                            nc.vector.tensor_scalar(
                                out=z, in0=pd, scalar1=EPS, scalar2=None,
                                op0=mybir.AluOpType.add,
                            )
                            nc.vector.reciprocal(out=z, in_=z)
                            # per-row absmax of the unnormalized pv
                            m0 = z_p.tile([128, 1], F32, tag="m0")
                            nc.vector.tensor_reduce(
                                out=m0, in_=pv0, axis=mybir.AxisListType.X,
                                op=mybir.AluOpType.max, apply_absolute_value=True,
                            )
                            m1 = z_p.tile([128, 1], F32, tag="m1")
                            nc.vector.tensor_reduce(
                                out=m1, in_=pv1, axis=mybir.AxisListType.X,
                                op=mybir.AluOpType.max, apply_absolute_value=True,
                            )
                            m = z_p.tile([128, 1], F32, tag="m")
                            nc.vector.scalar_tensor_tensor(
                                out=m, in0=m0, scalar=0.0, in1=m1,
                                op0=mybir.AluOpType.add, op1=mybir.AluOpType.max,
                            )
                            # dequant scale for the host: m * z / 127
                            sc = z_p.tile([128, 1], F32, tag="sc")
                            nc.vector.scalar_tensor_tensor(
                                out=sc, in0=m, scalar=1.0 / 127.0, in1=z,
                                op0=mybir.AluOpType.mult, op1=mybir.AluOpType.mult,
                            )
                            # quant multiplier 127 / m
                            inv = z_p.tile([128, 1], F32, tag="inv")
                            nc.vector.reciprocal(out=inv, in_=m)
                            inv127 = z_p.tile([128, 1], F32, tag="inv127")
                            nc.vector.tensor_scalar_mul(
                                out=inv127, in0=inv, scalar1=127.0
                            )
                            # explicit round-to-nearest: the f32->int8 cast
                            # truncates toward zero, so add 0.5*sign first
                            ob = ob_p.tile([128, 1024], mybir.dt.int8)
                            for s_, pv_ in ((0, pv0), (512, pv1)):
                                obf = ob_p.tile(
                                    [128, 512], F32, tag=f"obf{s_}", name=f"obf{s_}"
                                )
                                nc.vector.tensor_scalar_mul(
                                    out=obf, in0=pv_, scalar1=inv127
                                )
                                sgn = ob_p.tile(
                                    [128, 512], F32, tag=f"sg{s_}", name=f"sg{s_}"
                                )
                                nc.scalar.activation(
                                    out=sgn, in_=obf, func=Act.Sign
                                )
                                nc.vector.scalar_tensor_tensor(
                                    out=ob[:, s_ : s_ + 512], in0=sgn, scalar=0.5,
                                    in1=obf, op0=mybir.AluOpType.mult,
                                    op1=mybir.AluOpType.add,
                                )
                            r0 = (g * 4 + lc) * 128
                            nc.sync.dma_start(out=out[r0 : r0 + 128, :], in_=ob)
                            nc.sync.dma_start(
                                out=out_sc[r0 : r0 + 128, :], in_=sc
                            )
    if not nc.is_finalized():
        nc.finalize()
    return nc


def _get_nc(use_cc):
    if use_cc not in _NC_CACHE:
        _NC_CACHE[use_cc] = build_bass(use_cc)
    return _NC_CACHE[use_cc]


def _prep_inputs(x, W, use_cc):
    """Build per-core input maps (host-side shard + transpose + bf16 cast).

    The transpose runs on a uint16 view — numpy's transpose/copy kernels
    for custom dtypes (ml_dtypes.bfloat16) fall off the fast path.
    """
    w16 = W.astype(NPBF16)
    x16u = x.astype(NPBF16).view(np.uint16)

    if use_cc:
        # core c = (b, half); xt_h[p, k, n] = x[b, half*R + n, k*128 + p]
        xt_all = np.ascontiguousarray(
            x16u.reshape(B, 2, R, 8, 128).transpose(0, 1, 4, 3, 2)
        ).view(NPBF16)
        return [
            {
                "xt": xt_all[c // 2, c % 2],
                "w": w16[(c * 128) : (c + 1) * 128, :],
            }
            for c in range(NCORES)
        ]

    in_maps = []
    for c in range(NCORES):
        b, half = divmod(c, 2)
        own = x16u.reshape(B, 2, R, 1024)[b, half]
        sib = x16u.reshape(B, 2, R, 1024)[b, 1 - half]
        rows = np.concatenate([own, sib], axis=0)          # [8192, 1024] u16
        xt_h = np.ascontiguousarray(
            rows.reshape(2 * R, 8, 128).transpose(2, 1, 0)
        ).view(NPBF16)
        in_maps.append({"xt": xt_h, "w": w16})
    return in_maps


def kernel(x, W):
    global LAST_RESULTS
    from concourse.bass_utils import run_bass_kernel_spmd

    x = np.asarray(x, dtype=np.float32)
    W = np.asarray(W, dtype=np.float32)
    nc = _get_nc(USE_CC)
    in_maps = _prep_inputs(x, W, USE_CC)
    try:
        res = run_bass_kernel_spmd(
            nc, in_maps, core_ids=list(range(NCORES)), trace=TRACE
        )
    except ModuleNotFoundError:
        # NTFF profiling hook unavailable (axon client without antenv.axon_hooks)
        res = run_bass_kernel_spmd(
            nc, in_maps, core_ids=list(range(NCORES)), trace=False
        )
    LAST_RESULTS = res
    out = np.empty((B, L, D), dtype=np.float32)
    for c in range(NCORES):
        b, half = divmod(c, 2)
        o = res.results[c]["out"].astype(np.float32)
        o *= np.asarray(res.results[c]["out_sc"], dtype=np.float32)
        out[b, half * R : (half + 1) * R] = o
    return out


# revision 8
# speedup vs baseline: 1.6166x; 1.6166x over previous
"""Linear attention Bass kernel for Trainium2 (8 NeuronCores).

Problem: x [4, 8192, 1024] f32, W [1024, 3072] f32.
  qkv = x @ W; q,k,v = split(qkv); q,k = elu(.)+1
  KV = einsum('bld,blh->bhd', k, v); ksum = k.sum(1)
  Z = 1/(q.ksum + eps); V = einsum('bld,bhd,bl->blh', q, KV, Z)

Sharding: 8 cores, core c handles batch b=c//2, sequence half h=c%2
(4096 rows each).  KV / ksum reductions span the full batch sequence, so
the two cores of a pair AllReduce their partial KV^T [1024,1024] + ksum
(4.2 MB fp32) in-NEFF.

Under axon the dominant cost is host<->device PJRT traffic over the
tunnel (~75-110 MB/s, serialized), not on-chip time.  Traffic diet vs
the first working version (f32/bf16 everywhere, 385 MB, ~8 s):
  * W is shipped once as per-core [128, 3072] slices (6.3 MB total
    instead of 8x duplicated 50 MB) and AllGathered on-chip across all
    8 cores, then laid out into SBUF.
  * x is shipped int8 with a per-row f32 scale (rowmax/127): 32 MB
    instead of 64 MB bf16.  The dequant multiply rides the per-partition
    scale port of the DVE/Act engines after the qkv matmuls, which
    requires q to be computed in standard [row, dim] form (partition =
    row); phase 3 reloads q^T through transposing DMA reads.
  * the output is int8 with a per-row f32 dequant scale (quarters both
    the donated zero-output upload and the result download vs f32).
    V_row = z * pv_row with z > 0, so int8 = pv * (127/rowmax|pv|) and
    scale = rowmax|pv| * z / 127; the host computes i8 * scale.  The
    f32->int8 cast rounds to nearest on HW (measured).

Per-core dataflow (matmuls bf16 inputs, fp32 PSUM accumulation):
  phase 0: DMA w_slice -> SBUF -> internal DRAM; AllGather[0..7] ->
           full W [1024, 3072] on every core; 16 strided DMAs lay it
           out as wq_sb [128,8,1024] / wkv_sb [128,8,2048].
  phase 1: stream xT int8 tiles, upcast to bf16; per 128-row chunk:
           q = x@Wq in standard form, phi(scale)=elu(s*y)+1 via
           exp/max; q -> DRAM stash [lc,128,1024]; k,v same form with
           phi/scale; ksum accumulated in PSUM via ones-vector matmul.
  phase 2: KV^T[d,h] += k_tile^T-free matmul over all l chunks, h in two
           512 halves (PSUM = 8 banks per half); partial KV^T + ksum ->
           cc buffer; AllReduce over core pairs.
  phase 3: per 128-row chunk: transposing DMA loads q^T [d,k,l];
           V = (q^T)^T @ KV^T, denominator from ksum column matmul,
           z = 1/(den+eps), row absmax -> int8 quant, DMA out.
"""

import numpy as np
import ml_dtypes

import concourse.bass as bass
import concourse.tile as tile
from concourse import mybir
from concourse.bacc import Bacc

USE_CC = True
TRACE = False
LAST_RESULTS = None

B, L, D = 4, 8192, 1024
NCORES = 8
R = 4096              # rows per core
LT = 512              # l-tile width (columns of xT per tile)
N_LC = R // 128       # 32 row chunks
EPS = 1e-6

BF16 = mybir.dt.bfloat16
F32 = mybir.dt.float32
I8 = mybir.dt.int8
NPBF16 = ml_dtypes.bfloat16

_NC_CACHE = {}


def _emit_phi(nc, pool_e, out_bf, psum_in, width, scale=None):
    """out_bf (bf16) = elu(s*y)+1 = min(exp(s*y),1) + max(s*y,0).

    `scale` (per-partition AP or None) is the int8 dequant factor; the
    Act engine applies it inside exp, the DVE max-path multiplies after
    the max (s > 0 so the two commute).  Ops are emitted per 512-wide
    slice so each reads a single PSUM bank (one stop-matmul dep).
    """
    for s in range(0, width, 512):
        w = min(512, width - s)
        ps = psum_in[:, s : s + w]
        e = pool_e.tile([128, w], F32, tag=f"phi_e_{w}_{s}", name=f"e{w}_{s}")
        r = pool_e.tile([128, w], F32, tag=f"phi_r_{w}_{s}", name=f"r{w}_{s}")
        if scale is None:
            nc.scalar.activation(
                out=e, in_=ps, func=mybir.ActivationFunctionType.Exp
            )
            nc.vector.tensor_scalar(
                out=r, in0=ps, scalar1=0.0, scalar2=None, op0=mybir.AluOpType.max
            )
        else:
            nc.scalar.activation(
                out=e, in_=ps, func=mybir.ActivationFunctionType.Exp, scale=scale
            )
            nc.vector.tensor_scalar(
                out=r, in0=ps, scalar1=0.0, scalar2=scale,
                op0=mybir.AluOpType.max, op1=mybir.AluOpType.mult,
            )
        nc.vector.scalar_tensor_tensor(
            out=out_bf[:, s : s + w],
            in0=e,
            scalar=1.0,
            in1=r,
            op0=mybir.AluOpType.min,
            op1=mybir.AluOpType.add,
        )


def build_bass(use_cc=USE_CC):
    assert use_cc, "collective-free fallback was removed"
    nc = Bacc(trn_type="TRN2", num_devices=NCORES)

    xt = nc.dram_tensor("xt", [128, 8, R], I8, kind="ExternalInput")
    xsc = nc.dram_tensor("xsc", [N_LC, 128], F32, kind="ExternalInput")
    # each core brings 1/8 of W; AllGather reconstructs the full W
    w_slice = nc.dram_tensor("w", [128, 3072], BF16, kind="ExternalInput")
    w_in = nc.dram_tensor("w_in", [128, 3072], BF16)
    w_all = nc.dram_tensor("w_all", [1024, 3072], BF16)
    out = nc.dram_tensor("out", [R, 1024], I8, kind="ExternalOutput")
    out_sc = nc.dram_tensor("out_sc", [R, 1], F32, kind="ExternalOutput")

    q_dram = nc.dram_tensor("q_stash", [N_LC, 128, 1024], BF16)
    k_dram = nc.dram_tensor("k_stash", [N_LC, 128, 1024], BF16)
    v_dram = nc.dram_tensor("v_stash", [N_LC, 128, 1024], BF16)
    # row 128 of each [129, 1024] chunk holds ksum[m*128:(m+1)*128] in
    # cols 0:128 (rest zeros, harmlessly allreduced).
    cc_in = nc.dram_tensor("cc_in", [8, 129, 1024], F32)
    cc_out = nc.dram_tensor("cc_out", [8, 129, 1024], F32)

    mm = nc.tensor.matmul
    Act = mybir.ActivationFunctionType

    with tile.TileContext(nc) as tc:
        with tc.tile_pool(name="consts", bufs=1) as consts:
            # ---------------- phase 0: W distribution ---------------------
            w_sb = consts.tile([128, 3072], BF16)
            nc.sync.dma_start(out=w_sb, in_=w_slice[:])
            nc.sync.dma_start(out=w_in[:], in_=w_sb)
            nc.gpsimd.collective_compute(
                "AllGather",
                mybir.AluOpType.bypass,
                replica_groups=[list(range(NCORES))],
                ins=[w_in[:]],
                outs=[w_all[:]],
            )
            wq_sb = consts.tile([128, 8, 1024], BF16)
            wkv_sb = consts.tile([128, 8, 2048], BF16)
            for k in range(8):
                nc.sync.dma_start(
                    out=wq_sb[:, k, :], in_=w_all[k * 128 : (k + 1) * 128, 0:1024]
                )
                nc.sync.dma_start(
                    out=wkv_sb[:, k, :],
                    in_=w_all[k * 128 : (k + 1) * 128, 1024:3072],
                )
            ones_sb = consts.tile([128, 1], BF16)
            nc.vector.memset(ones_sb, 1.0)
            xsc_sb = consts.tile([128, N_LC], F32)
            nc.sync.dma_start(out=xsc_sb, in_=xsc[:].transpose([1, 0]))

            # ---------------- phase 1: qkv + phi + stashes + ksum ---------
            with (
                tc.tile_pool(name="xt_p", bufs=3) as xt_p,
                tc.tile_pool(name="xbf_p", bufs=2) as xbf_p,
                tc.tile_pool(name="qo_p", bufs=2) as qo_p,
                tc.tile_pool(name="e_p", bufs=4) as e_p,
                tc.tile_pool(name="kt_p", bufs=3) as kt_p,
                tc.tile_pool(name="vt_p", bufs=3) as vt_p,
                tc.tile_pool(name="q_ps_p", bufs=1, space="PSUM") as q_ps_p,
                tc.tile_pool(name="kv_ps_p", bufs=1, space="PSUM") as kv_ps_p,
                tc.tile_pool(name="ks_ps_p", bufs=1, space="PSUM") as ks_ps_p,
            ):
                ksum_ps = [
                    ks_ps_p.tile([1, 512], F32, tag=f"ks{h}", name=f"ks{h}")
                    for h in range(2)
                ]

                for t in range(R // LT):
                    xt_i8 = xt_p.tile([128, 8, LT], I8)
                    nc.sync.dma_start(
                        out=xt_i8, in_=xt[:, :, t * LT : (t + 1) * LT]
                    )
                    xt_bf = xbf_p.tile([128, 8, LT], BF16)
                    nc.vector.tensor_copy(out=xt_bf, in_=xt_i8)

                    for j in range(LT // 128):
                        lc = t * 4 + j
                        ssc = xsc_sb[:, lc : lc + 1]
                        chunk = slice(j * 128, (j + 1) * 128)

                        # ---- q in standard [row, dim] form ----
                        q_std = qo_p.tile([128, 1024], BF16)
                        for h in range(2):
                            pq = q_ps_p.tile(
                                [128, 512], F32, tag=f"pq{h}", name=f"pq{h}"
                            )
                            for k in range(8):
                                mm(
                                    pq,
                                    lhsT=xt_bf[:, k, chunk],
                                    rhs=wq_sb[:, k, h * 512 : (h + 1) * 512],
                                    start=(k == 0),
                                    stop=(k == 7),
                                )
                            _emit_phi(
                                nc, e_p, q_std[:, h * 512 : (h + 1) * 512],
                                pq, 512, scale=ssc,
                            )
                        nc.sync.dma_start(out=q_dram[lc], in_=q_std)

                        # ---- k, v ----
                        pkv = [
                            kv_ps_p.tile(
                                [128, 512], F32, tag=f"pkv{n}", name=f"pkv{n}"
                            )
                            for n in range(4)
                        ]
                        for k in range(8):
                            lhsT = xt_bf[:, k, chunk]
                            for n in range(4):
                                mm(
                                    pkv[n],
                                    lhsT=lhsT,
                                    rhs=wkv_sb[:, k, n * 512 : (n + 1) * 512],
                                    start=(k == 0),
                                    stop=(k == 7),
                                )
                        kt = kt_p.tile([128, 1024], BF16)
                        for s in range(2):
                            _emit_phi(
                                nc, e_p, kt[:, s * 512 : (s + 1) * 512],
                                pkv[s], 512, scale=ssc,
                            )
                        vt = vt_p.tile([128, 1024], BF16)
                        for s in range(2):
                            nc.vector.tensor_scalar_mul(
                                out=vt[:, s * 512 : (s + 1) * 512],
                                in0=pkv[2 + s],
                                scalar1=ssc,
                            )
                        nc.sync.dma_start(out=k_dram[lc], in_=kt)
                        nc.sync.dma_start(out=v_dram[lc], in_=vt)
                        for h in range(2):
                            mm(
                                ksum_ps[h],
                                lhsT=ones_sb,
                                rhs=kt[:, h * 512 : (h + 1) * 512],
                                start=(lc == 0),
                                stop=(lc == N_LC - 1),
                            )

                # stash ksum (psum) to DRAM before phase-1 psum pools close
                ks_sb = consts.tile([1, 1024], F32)
                for h in range(2):
                    nc.vector.tensor_copy(
                        out=ks_sb[:, h * 512 : (h + 1) * 512], in_=ksum_ps[h]
                    )
                for m in range(8):
                    nc.sync.dma_start(
                        out=cc_in[m, 128, 0:128],
                        in_=ks_sb[0:1, m * 128 : (m + 1) * 128],
                    )

            # ---------------- phase 2: KV^T accumulation ------------------
            with tc.tile_pool(name="p23", bufs=1) as p23:
                with (
                    tc.tile_pool(name="k2_p", bufs=6) as k2_p,
                    tc.tile_pool(name="v2_p", bufs=6) as v2_p,
                    tc.tile_pool(name="kvt_ps_p", bufs=1, space="PSUM") as kvt_ps_p,
                ):
                    for half in range(2):
                        kvt_ps = [
                            kvt_ps_p.tile(
                                [128, 512], F32, tag=f"kvt{m}", name=f"kvt{m}"
                            )
                            for m in range(8)
                        ]
                        for lc in range(N_LC):
                            kt2 = k2_p.tile([128, 1024], BF16)
                            nc.sync.dma_start(out=kt2, in_=k_dram[lc])
                            vt2 = v2_p.tile([128, 512], BF16)
                            nc.sync.dma_start(
                                out=vt2,
                                in_=v_dram[lc][:, half * 512 : (half + 1) * 512],
                            )
                            for m in range(8):
                                mm(
                                    kvt_ps[m],
                                    lhsT=kt2[:, m * 128 : (m + 1) * 128],
                                    rhs=vt2,
                                    start=(lc == 0),
                                    stop=(lc == N_LC - 1),
                                )
                        for m in range(8):
                            kvs = k2_p.tile(
                                [128, 512], F32, tag="kvs", name=f"kvs{half}_{m}"
                            )
                            nc.scalar.activation(
                                out=kvs, in_=kvt_ps[m], func=Act.Copy
                            )
                            nc.sync.dma_start(
                                out=cc_in[m, 0:128, half * 512 : (half + 1) * 512],
                                in_=kvs,
                            )

                nc.gpsimd.collective_compute(
                    "AllReduce",
                    mybir.AluOpType.add,
                    replica_groups=[[0, 1], [2, 3], [4, 5], [6, 7]],
                    ins=[cc_in[:]],
                    outs=[cc_out[:]],
                )

                # ---------------- phase 3: output -------------------------
                with (
                    tc.tile_pool(name="p3", bufs=1) as p3,
                    tc.tile_pool(name="qt_p", bufs=2) as qt_p,
                    tc.tile_pool(name="ob_p", bufs=3) as ob_p,
                    tc.tile_pool(name="z_p", bufs=4) as z_p,
                    tc.tile_pool(name="pv_ps_p", bufs=2, space="PSUM") as pv_ps_p,
                    tc.tile_pool(name="pd_ps_p", bufs=2, space="PSUM") as pd_ps_p,
                ):
                    kvt_f = p3.tile([128, 8, 1024], F32)
                    for m in range(8):
                        nc.sync.dma_start(
                            out=kvt_f[:, m, :], in_=cc_out[m, 0:128, :]
                        )
                    kvt_bf = p3.tile([128, 8, 1024], BF16)
                    for m in range(8):
                        nc.vector.tensor_copy(
                            out=kvt_bf[:, m, :], in_=kvt_f[:, m, :]
                        )
                    ksum_f = p3.tile([128, 8], F32)
                    for m in range(8):
                        nc.sync.dma_start(
                            out=ksum_f[:, m : m + 1], in_=cc_out[m, 128, 0:128]
                        )
                    ksum_b = p3.tile([128, 8], BF16)
                    for m in range(8):
                        nc.vector.tensor_copy(
                            out=ksum_b[:, m : m + 1], in_=ksum_f[:, m : m + 1]
                        )

                    for lc in range(N_LC):
                        # transposing loads: q^T [d_off, k, l] from the
                        # standard-form stash [l, k*128+d_off]
                        qT = qt_p.tile([128, 8, 128], BF16)
                        for k in range(8):
                            nc.sync.dma_start(
                                out=qT[:, k, :],
                                in_=q_dram[
                                    lc, :, k * 128 : (k + 1) * 128
                                ].transpose([1, 0]),
                            )
                        pv0 = pv_ps_p.tile([128, 512], F32, tag="pv0")
                        pv1 = pv_ps_p.tile([128, 512], F32, tag="pv1")
                        pd = pd_ps_p.tile([128, 1], F32)
                        for k in range(8):
                            lhsT = qT[:, k, :]
                            st, sp = (k == 0), (k == 7)
                            mm(pv0, lhsT=lhsT, rhs=kvt_bf[:, k, 0:512],
                               start=st, stop=sp)
                            mm(pv1, lhsT=lhsT, rhs=kvt_bf[:, k, 512:1024],
                               start=st, stop=sp)
                            mm(pd, lhsT=lhsT, rhs=ksum_b[:, k : k + 1],
                               start=st, stop=sp)
                        z = z_p.tile([128, 1], F32)
                        nc.vector.tensor_scalar(
                            out=z, in0=pd, scalar1=EPS, scalar2=None,
                            op0=mybir.AluOpType.add,
                        )
                        nc.vector.reciprocal(out=z, in_=z)
                        # per-row absmax of the unnormalized pv
                        m0 = z_p.tile([128, 1], F32, tag="m0")
                        nc.vector.tensor_reduce(
                            out=m0, in_=pv0, axis=mybir.AxisListType.X,
                            op=mybir.AluOpType.max, apply_absolute_value=True,
                        )
                        m1 = z_p.tile([128, 1], F32, tag="m1")
                        nc.vector.tensor_reduce(
                            out=m1, in_=pv1, axis=mybir.AxisListType.X,
                            op=mybir.AluOpType.max, apply_absolute_value=True,
                        )
                        m = z_p.tile([128, 1], F32, tag="m")
                        nc.vector.scalar_tensor_tensor(
                            out=m, in0=m0, scalar=0.0, in1=m1,
                            op0=mybir.AluOpType.add, op1=mybir.AluOpType.max,
                        )
                        # dequant scale for the host: m * z / 127
                        sc = z_p.tile([128, 1], F32, tag="sc")
                        nc.vector.scalar_tensor_tensor(
                            out=sc, in0=m, scalar=1.0 / 127.0, in1=z,
                            op0=mybir.AluOpType.mult, op1=mybir.AluOpType.mult,
                        )
                        # quant multiplier 127 / m
                        inv = z_p.tile([128, 1], F32, tag="inv")
                        nc.vector.reciprocal(out=inv, in_=m)
                        inv127 = z_p.tile([128, 1], F32, tag="inv127")
                        nc.vector.tensor_scalar_mul(
                            out=inv127, in0=inv, scalar1=127.0
                        )
                        ob = ob_p.tile([128, 1024], I8)
                        nc.vector.tensor_scalar_mul(
                            out=ob[:, 0:512], in0=pv0, scalar1=inv127
                        )
                        nc.vector.tensor_scalar_mul(
                            out=ob[:, 512:1024], in0=pv1, scalar1=inv127
                        )
                        r0 = lc * 128
                        nc.sync.dma_start(out=out[r0 : r0 + 128, :], in_=ob)
                        nc.sync.dma_start(out=out_sc[r0 : r0 + 128, :], in_=sc)
    if not nc.is_finalized():
        nc.finalize()
    return nc


def _get_nc(use_cc):
    if use_cc not in _NC_CACHE:
        _NC_CACHE[use_cc] = build_bass(use_cc)
    return _NC_CACHE[use_cc]


def _prep_inputs(x, W, use_cc=True):
    """Per-core input maps: int8 row-quantized x^T shards + W slices."""
    w16 = W.astype(NPBF16)
    s = np.abs(x).max(axis=-1)                    # [B, L] rowmax
    np.maximum(s, 1e-12, out=s)
    xq = np.rint(x * (127.0 / s)[..., None]).astype(np.int8)
    # xt[p, k, n] = xq[b, half*R + n, k*128 + p]
    xt_all = np.ascontiguousarray(
        xq.reshape(B, 2, R, 8, 128).transpose(0, 1, 4, 3, 2)
    )
    ssc_all = (s / 127.0).astype(np.float32).reshape(B, 2, N_LC, 128)
    return [
        {
            "xt": xt_all[c // 2, c % 2],
            "xsc": ssc_all[c // 2, c % 2],
            "w": w16[(c * 128) : (c + 1) * 128, :],
        }
        for c in range(NCORES)
    ]


def kernel(x, W):
    global LAST_RESULTS
    from concourse.bass_utils import run_bass_kernel_spmd

    x = np.asarray(x, dtype=np.float32)
    W = np.asarray(W, dtype=np.float32)
    nc = _get_nc(True)
    in_maps = _prep_inputs(x, W)
    try:
        res = run_bass_kernel_spmd(
            nc, in_maps, core_ids=list(range(NCORES)), trace=TRACE
        )
    except ModuleNotFoundError:
        # NTFF profiling hook unavailable (axon client without antenv.axon_hooks)
        res = run_bass_kernel_spmd(
            nc, in_maps, core_ids=list(range(NCORES)), trace=False
        )
    LAST_RESULTS = res
    out = np.empty((B, L, D), dtype=np.float32)
    for c in range(NCORES):
        b, half = divmod(c, 2)
        o = res.results[c]["out"].astype(np.float32)
        o *= np.asarray(res.results[c]["out_sc"], dtype=np.float32)
        out[b, half * R : (half + 1) * R] = o
    return out
